# revision 53
# baseline (speedup 1.0000x reference)
"""Bass/Trainium2 kernel for nn_EnhancedOrthogonal (complex column
orthogonalization, 2 iterations).

Single-AllReduce restructure: with G = x^H x the *global* Gram known on every
core after one AllReduce, both iterations' corrections are pure R x R algebra
(G1 = B0^H G0 B0 etc.), so the kernel is:

  pass 1 (per core): partial Gram of the raw M-shard, 3 real bf16 matmul
      products (Gre = Xr^T Xr + Xi^T Xi, T = Xr^T Xi; Gim = T - T^T is formed
      after the reduce since transpose commutes with the sum).
  AllReduce: two 4MB f32 reduces (Gre, T) so the first overlaps T's compute.
  local chain (replicated on all cores): G0 = D0^-1 G D0^-1, B0 = I - 0.5
      offdiag(G0); T1 = G0 B0; n1 = sqrt(diag(B0^H T1) + eps); M1 = B0^H T1;
      G1 = D1^-1 M1 D1^-1; B1 = I - 0.5 offdiag(G1); n2 from G1 B1;
      C = D0^-1 B0 D1^-1 B1 D2^-1  (4 complex R x R matmuls, plain 4-product
      complex arithmetic so no extra Karatsuba components are stored).
  pass 2 (per core): Y = x_c @ C via Karatsuba (3 bf16 products), interleave,
      store.

x stays SBUF-resident in bf16 (the M-shard is loaded from HBM exactly once);
norm reductions are carried in fp32. Chain matrices live in six persistent
bf16 [R, R]-component buffers reused via tag aliasing:
    g:  G0 -> M1/G1 -> C.re/C.im
    b0: B0' -> A -> C.sum
    t1: (T^T scratch, T1) -> B1'/E
"""

from contextlib import ExitStack

import numpy as np

import concourse.bacc as bacc
import concourse.bass as bass
import concourse.mybir as mybir
import concourse.tile as tile
from concourse.bass import ds, ts
from concourse.bass_utils import run_bass_kernel_spmd
from concourse.masks import make_identity

P = 128
M_FULL = 16384
R = 1024
N_CORES = 8
MS = M_FULL // N_CORES          # 2048 rows per core
NMT = MS // P                   # 16 m-tiles per core
NAB = R // P                    # 8 column blocks of 128
NCH = 2                         # 512-wide column chunks
CW = R // NCH                   # 512
EPS = 1e-8
F32 = mybir.dt.float32
BF16 = mybir.dt.bfloat16

_CACHE = {}


def _finalize(nc):
    nc.compile()
    # Tile's deferred wait assignment can leave multi-wait DMAs that the
    # in-compile generate_event_semaphores pass missed; DMA instructions
    # support a single HW wait slot, so re-split (and re-codegen) here.
    nc.generate_event_semaphores()
    nc.codegen_inst_isa_subclasses()


def _build_nc(debug_stage: int = 99, reps: int = 1, single_core: bool = False,
              timing: bool = False):
    nc = bacc.Bacc("TRN2", target_bir_lowering=False, debug=False,
                   num_devices=1 if single_core else N_CORES)
    if timing:
        xin = nc.dram_tensor("x", [P, 4, 2], F32, kind="ExternalInput")
    else:
        xin = nc.dram_tensor("x", [MS, R, 2], F32, kind="ExternalInput")
    out = nc.dram_tensor("out", [MS, R, 2], F32, kind="ExternalOutput")

    with tile.TileContext(nc) as tc, ExitStack() as ctx:
        consts = ctx.enter_context(tc.tile_pool(name="consts", bufs=1))
        persist = ctx.enter_context(tc.tile_pool(name="persist", bufs=1))
        dram = ctx.enter_context(tc.tile_pool(name="dram", bufs=1, space="DRAM"))

        identity = consts.tile([P, P], F32)
        make_identity(nc, identity)
        ident_bf = consts.tile([P, P], BF16)
        nc.vector.tensor_copy(ident_bf, identity)
        omI = consts.tile([P, P], F32)  # 1 - I
        nc.vector.tensor_scalar(out=omI, in0=identity, scalar1=-1.0,
                                scalar2=1.0, op0=mybir.AluOpType.mult,
                                op1=mybir.AluOpType.add)
        omI_bf = consts.tile([P, P], BF16)
        nc.vector.tensor_copy(omI_bf, omI)
        ones1 = consts.tile([1, P], F32)
        nc.vector.memset(ones1, 1.0)
        onesc = consts.tile([P, 1], F32)
        nc.vector.memset(onesc, 1.0)

        if timing:
            x = dram.tile([MS, R, 2], F32, tag="xtim", name="xtim")
        else:
            x = xin

        # persistent SBUF: the bf16 M-shard, components separated. The six
        # chain component buffers (tags g_r/g_i/b0_r/b0_i/t1_r/t1_i, reuse
        # map in the module docstring) are allocated per rep inside chain(),
        # in usage order, so every generation change is explicit.
        qr_all = persist.tile([P, NMT, R], BF16, tag="qr", name="qr")
        qi_all = persist.tile([P, NMT, R], BF16, tag="qi", name="qi")

        def bcast_row(row, dst, psum_pool, tag):
            """dst[p, :] = row[0, :] for all p, via K=1 outer-product matmul."""
            for n in range(NCH):
                v_ps = psum_pool.tile([P, CW], F32, tag=tag, name=tag)
                nc.tensor.matmul(v_ps, ones1, row[:, ds(n * CW, CW)],
                                 start=True, stop=True)
                nc.vector.tensor_copy(dst[:, ds(n * CW, CW)], v_ps)

        def colsum_norms(acc, pool, tag, want_w=True, want_f32=True):
            """acc [P, R] f32 -> (vfull f32, vfull bf16, w [P, NAB] f32):
            cross-partition colsum + EPS -> 1/sqrt, in column-broadcast (both
            dtypes) and per-partition-scalar forms. The [1, R] row buffer is
            shared (tag "chrow") with build_g0b0's vrow0 -- all uses are
            sequential."""
            with tc.tile_pool(name=f"{tag}_ps", bufs=1, space="PSUM") as psp:
                row = pool.tile([1, R], F32, tag="chrow")
                for n in range(NCH):
                    nsl = ds(n * CW, CW)
                    rp = psp.tile([1, CW], F32, tag=f"{tag}_rp",
                                  name=f"{tag}_rp")
                    nc.tensor.matmul(rp, onesc, acc[:, nsl], start=True,
                                     stop=True)
                    nc.vector.tensor_copy(row[:, nsl], rp)
                nc.vector.tensor_scalar_add(row, row, EPS)
                nc.vector.reciprocal(row, row)
                nc.scalar.sqrt(row, row)
                vf = None
                if want_f32:
                    vf = pool.tile([P, R], F32, tag=f"{tag}_vf",
                                   name=f"{tag}_vf")
                vfb = pool.tile([P, R], BF16, tag=f"{tag}_vfb")
                for n in range(NCH):
                    nsl = ds(n * CW, CW)
                    v_ps = psp.tile([P, CW], F32, tag=f"{tag}_vps",
                                    name=f"{tag}_vps")
                    nc.tensor.matmul(v_ps, ones1, row[:, nsl], start=True,
                                     stop=True)
                    if want_f32:
                        nc.vector.tensor_copy(vf[:, nsl], v_ps)
                    nc.scalar.copy(vfb[:, nsl], v_ps)
                w = None
                if want_w:
                    nvd = dram.tile([NAB, P], F32, tag=f"{tag}_nvd",
                                    name=f"{tag}_nvd")
                    nc.sync.dma_start(nvd, row)
                    w = pool.tile([P, NAB], F32, tag=f"{tag}_w")
                    for a in range(NAB):
                        nc.sync.dma_start(w[:, ds(a, 1)], nvd[a, :])
            return vf, vfb, w

        # ---------------- pass 0: load + deinterleave ----------------
        def load_x():
            with tc.tile_pool(name="ldx", bufs=3) as ldp:
                for m in range(NMT):
                    xt = ldp.tile([P, R, 2], F32, tag="xt")
                    nc.sync.dma_start(xt, x[ts(m, P), :, :])
                    nc.vector.tensor_copy(qr_all[:, m, :], xt[:, :, 0])
                    nc.scalar.copy(qi_all[:, m, :], xt[:, :, 1])

        # ---------------- pass 1: partial Gram ----------------
        def gram_pass(gre_dst, t_dst, ar_gre):
            with (
                tc.tile_pool(name="gr_ps", bufs=1, space="PSUM") as psp,
                tc.tile_pool(name="gr_out", bufs=3) as gop,
            ):
                # Gre = Xr^T Xr + Xi^T Xi
                for c in range(NCH):
                    csl = ds(c * CW, CW)
                    ps = [psp.tile([P, CW], F32, tag=f"g{t}", name=f"g{t}")
                          for t in range(NAB)]
                    for m in range(NMT):
                        for a in range(NAB):
                            asl = ds(a * P, P)
                            nc.tensor.matmul(ps[a], qr_all[:, m, asl],
                                             qr_all[:, m, csl],
                                             start=(m == 0), stop=False)
                            nc.tensor.matmul(ps[a], qi_all[:, m, asl],
                                             qi_all[:, m, csl],
                                             start=False, stop=(m == NMT - 1))
                    for a in range(NAB):
                        g_sb = gop.tile([P, CW], F32, tag="gsb")
                        nc.vector.tensor_copy(g_sb, ps[a])
                        nc.sync.dma_start(gre_dst[ts(a, P), csl], g_sb)
                ar_gre()
                # T = Xr^T Xi
                for c in range(NCH):
                    csl = ds(c * CW, CW)
                    ps = [psp.tile([P, CW], F32, tag=f"g{t}", name=f"g{t}")
                          for t in range(NAB)]
                    for m in range(NMT):
                        for a in range(NAB):
                            nc.tensor.matmul(ps[a], qr_all[:, m, ds(a * P, P)],
                                             qi_all[:, m, csl],
                                             start=(m == 0),
                                             stop=(m == NMT - 1))
                    for a in range(NAB):
                        g_sb = gop.tile([P, CW], F32, tag="gsb")
                        nc.vector.tensor_copy(g_sb, ps[a])
                        nc.sync.dma_start(t_dst[ts(a, P), csl], g_sb)

        # ---------------- post-AR: G0, B0' ----------------
        def build_g0b0(gre_src, t_src, seq, w0, g_r, g_i, b0_r, b0_i):
            """Fill g (G0), b0 (B0') bf16 from the reduced Gre/T; w0 [P, NAB]
            f32 gets n0inv. Uses t1_i's buffer as T^T scratch."""
            with (
                tc.tile_pool(name="pg", bufs=1) as pool,
                tc.tile_pool(name="pg_ps", bufs=2, space="PSUM") as psp,
            ):
                # diag blocks -> d0 -> n0inv forms
                dsq = seq.tile([P, NAB], F32, tag="pg_dsq")
                for a in range(NAB):
                    gd = pool.tile([P, P], F32, tag="pg_gd")
                    nc.sync.dma_start(gd, gre_src[ts(a, P), ts(a, P)])
                    nc.vector.tensor_mul(gd, gd, identity)
                    nc.vector.tensor_reduce(dsq[:, ds(a, 1)], gd,
                                            mybir.AxisListType.X,
                                            mybir.AluOpType.add)
                nc.vector.tensor_scalar_add(w0, dsq, EPS)
                nc.vector.reciprocal(w0, w0)
                nc.scalar.sqrt(w0, w0)
                vT_ps = psp.tile([NAB, P], F32, tag="pg_vT", name="pg_vT")
                nc.tensor.transpose(vT_ps, w0, identity)
                vT = pool.tile([NAB, P], F32, tag="pg_vTs")
                nc.vector.tensor_copy(vT, vT_ps)
                vrow0 = seq.tile([1, R], F32, tag="chrow")
                nc.sync.dma_start(vrow0, vT)
                vfull0 = pool.tile([P, R], F32, tag="pg_vfull0")
                vfull0b = seq.tile([P, R], BF16, tag="pg_vfull0b")
                for n in range(NCH):
                    nsl = ds(n * CW, CW)
                    v_ps = psp.tile([P, CW], F32, tag="pg_vps", name="pg_vps")
                    nc.tensor.matmul(v_ps, ones1, vrow0[:, nsl], start=True,
                                     stop=True)
                    nc.vector.tensor_copy(vfull0[:, nsl], v_ps)
                    nc.scalar.copy(vfull0b[:, nsl], v_ps)
                # G0r / B0'r per block
                for a in range(NAB):
                    gb = pool.tile([P, R], F32, tag="pg_big")
                    nc.sync.dma_start(gb, gre_src[ts(a, P), :])
                    nc.vector.tensor_mul(gb, gb, vfull0)
                    nc.vector.tensor_scalar_mul(g_r[:, a, :], gb,
                                                w0[:, ds(a, 1)])
                    nc.vector.tensor_scalar_mul(b0_r[:, a, :], g_r[:, a, :],
                                                -0.5)
                    bd = b0_r[:, a, ts(a, P)]
                    nc.vector.tensor_mul(bd, bd, omI_bf)
                    nc.vector.tensor_add(bd, bd, ident_bf)
                # T^T into t1_i's buffer (bf16 scratch), streaming T blocks
                tt = persist.tile([P, NAB, R], BF16, tag="t1_i", name="tt")
                for b in range(NAB):
                    tb = pool.tile([P, R], F32, tag="pg_big")
                    nc.sync.dma_start(tb, t_src[ts(b, P), :])
                    for g in range(NCH):
                        tp = psp.tile([P, 4, P], F32, tag="pg_tp", name="tp")
                        for k in range(4):
                            nc.tensor.transpose(tp[:, k, :],
                                                tb[:, ts(4 * g + k, P)],
                                                identity)
                        nc.scalar.copy(tt[:, ds(4 * g, 4), ts(b, P)], tp)
                # Gim row-blocks = T[a, :] - T^T[a, :], scaled -> g_i, b0_i
                for a in range(NAB):
                    tb = pool.tile([P, R], F32, tag="pg_big")
                    nc.sync.dma_start(tb, t_src[ts(a, P), :])
                    tbb = pool.tile([P, R], BF16, tag="pg_tbb")
                    nc.vector.tensor_copy(tbb, tb)
                    gi = pool.tile([P, R], BF16, tag="pg_gi")
                    nc.vector.tensor_sub(gi, tbb, tt[:, a, :])
                    nc.vector.tensor_mul(gi, gi, vfull0b)
                    nc.vector.tensor_scalar_mul(g_i[:, a, :], gi,
                                                w0[:, ds(a, 1)])
                    nc.vector.tensor_scalar_mul(b0_i[:, a, :], g_i[:, a, :],
                                                -0.5)
            return vfull0b

        # ---------------- complex R x R matmul helper ----------------
        def cmm(lr, li, rr, ri, flush, psp):
            """out = L^H @ R for the *stored* L components (PE matmul
            contracts over the partition dim, i.e. computes stored^T @ rhs).
            Every chain product is expressible this way: G0/G1 are Hermitian
            (G^H = G), M1 = B0'^H T1 directly, and C = A E = Ahat^H E with
            Ahat = D1^-1 B0' D0^-1 stored instead of A.

            flush(xb, c, p1, p2, p3) receives PSUM tiles holding
            p1 = Lr^T Rr + Li^T Ri = Zr; p2 = Lr^T Ri; p3 = Li^T Rr
            (Zi = p2 - p3)."""
            for xb in range(NAB):
                xsl = ds(xb * P, P)
                for c in range(NCH):
                    csl = ds(c * CW, CW)
                    ps1 = psp.tile([P, CW], F32, tag="mm1", name="mm1")
                    ps2 = psp.tile([P, CW], F32, tag="mm2", name="mm2")
                    ps3 = psp.tile([P, CW], F32, tag="mm3", name="mm3")
                    for k in range(NAB):
                        first, last = k == 0, k == NAB - 1
                        nc.tensor.matmul(ps1, lr[:, k, xsl], rr[:, k, csl],
                                         start=first, stop=False)
                        nc.tensor.matmul(ps1, li[:, k, xsl], ri[:, k, csl],
                                         start=False, stop=last)
                        nc.tensor.matmul(ps2, lr[:, k, xsl], ri[:, k, csl],
                                         start=first, stop=last)
                        nc.tensor.matmul(ps3, li[:, k, xsl], rr[:, k, csl],
                                         start=first, stop=last)
                    flush(xb, c, ps1, ps2, ps3)

        # ---------------- the local chain ----------------
        def chain(gre_src, t_src):
            with (
                tc.tile_pool(name="ch", bufs=2) as pool,
                tc.tile_pool(name="ch1", bufs=1) as pool1,
                tc.tile_pool(name="ch_seq", bufs=1) as seq,
            ):
                w0 = seq.tile([P, NAB], F32, tag="ch_w0")
                g_r = persist.tile([P, NAB, R], BF16, tag="g_r", name="g_r")
                g_i = persist.tile([P, NAB, R], BF16, tag="g_i", name="g_i")
                b0_r = persist.tile([P, NAB, R], BF16, tag="b0_r",
                                    name="b0_r")
                b0_i = persist.tile([P, NAB, R], BF16, tag="b0_i",
                                    name="b0_i")
                vfull0b = build_g0b0(gre_src, t_src, seq, w0, g_r, g_i,
                                     b0_r, b0_i)
                # fence: T1's flush writes the t1 buffers, whose previous
                # generation (the T^T scratch) is still being read by the
                # Gim ops above; don't let the scheduler hoist them.
                tc.no_sync_barrier()
                t1_r = persist.tile([P, NAB, R], BF16, tag="t1_r",
                                    name="t1_r")
                t1_i = persist.tile([P, NAB, R], BF16, tag="t1_i",
                                    name="t1_i")
                ctx2 = tc.tile_pool(name="ch_ps", bufs=2, space="PSUM")
                psp = ctx2.__enter__()

                # T1 = G0 @ B0', accumulate n1 (acc buffer shared with n2 --
                # uses are sequential)
                n1acc = seq.tile([P, R], F32, tag="chacc")

                def fl_t1(xb, c, ps1, ps2, ps3):
                    csl = ds(c * CW, CW)
                    tr = t1_r[:, xb, csl]
                    nc.scalar.copy(tr, ps1)
                    s3 = pool1.tile([P, CW], F32, tag="chc")
                    nc.scalar.copy(s3, ps3)
                    ti = t1_i[:, xb, csl]
                    nc.vector.tensor_sub(ti, ps2, s3)
                    pr = pool.tile([P, CW], F32, tag="chp")
                    nc.vector.tensor_mul(pr, tr, b0_r[:, xb, csl])
                    pi = pool.tile([P, CW], F32, tag="chq")
                    nc.vector.tensor_mul(pi, ti, b0_i[:, xb, csl])
                    nc.vector.tensor_add(pr, pr, pi)
                    if xb == 0:
                        nc.vector.tensor_copy(n1acc[:, csl], pr)
                    else:
                        nc.vector.tensor_add(n1acc[:, csl], n1acc[:, csl], pr)

                cmm(g_r, g_i, b0_r, b0_i, fl_t1, psp)
                vfull1, vfull1b, w1 = colsum_norms(n1acc, seq, "ch_n1")

                # M1 = B0'^H T1 ; G1 = D1^-1 M1 D1^-1 overwrites g
                def fl_m1(xb, c, ps1, ps2, ps3):
                    csl = ds(c * CW, CW)
                    tmp = pool.tile([P, CW], F32, tag="chp")
                    nc.vector.tensor_mul(tmp, ps1, vfull1[:, csl])
                    nc.vector.tensor_scalar_mul(g_r[:, xb, csl], tmp,
                                                w1[:, ds(xb, 1)])
                    s3 = pool1.tile([P, CW], F32, tag="chc")
                    nc.scalar.copy(s3, ps3)
                    nc.vector.tensor_sub(tmp, ps2, s3)
                    nc.vector.tensor_mul(tmp, tmp, vfull1[:, csl])
                    nc.vector.tensor_scalar_mul(g_i[:, xb, csl], tmp,
                                                w1[:, ds(xb, 1)])

                cmm(b0_r, b0_i, t1_r, t1_i, fl_m1, psp)

                # fence: B1' overwrites t1, still being read by M1 above
                tc.no_sync_barrier()
                # B1' = I - 0.5 offdiag(G1) overwrites t1
                b1_r = persist.tile([P, NAB, R], BF16, tag="t1_r",
                                    name="b1_r")
                b1_i = persist.tile([P, NAB, R], BF16, tag="t1_i",
                                    name="b1_i")
                for a in range(NAB):
                    nc.vector.tensor_scalar_mul(b1_r[:, a, :], g_r[:, a, :],
                                                -0.5)
                    bd = b1_r[:, a, ts(a, P)]
                    nc.vector.tensor_mul(bd, bd, omI_bf)
                    nc.vector.tensor_add(bd, bd, ident_bf)
                    nc.vector.tensor_scalar_mul(b1_i[:, a, :], g_i[:, a, :],
                                                -0.5)

                # T2 = G1 @ B1' -> n2 only (no store)
                n2acc = seq.tile([P, R], F32, tag="chacc")

                def fl_t2(xb, c, ps1, ps2, ps3):
                    csl = ds(c * CW, CW)
                    t2r = pool1.tile([P, CW], BF16, tag="chr")
                    nc.scalar.copy(t2r, ps1)
                    s3 = pool1.tile([P, CW], F32, tag="chc")
                    nc.scalar.copy(s3, ps3)
                    t2i = pool1.tile([P, CW], BF16, tag="chs")
                    nc.vector.tensor_sub(t2i, ps2, s3)
                    tmp = pool.tile([P, CW], F32, tag="chp")
                    nc.vector.tensor_mul(tmp, t2r, b1_r[:, xb, csl])
                    tmp2 = pool.tile([P, CW], F32, tag="chq")
                    nc.vector.tensor_mul(tmp2, t2i, b1_i[:, xb, csl])
                    nc.vector.tensor_add(tmp, tmp, tmp2)
                    if xb == 0:
                        nc.vector.tensor_copy(n2acc[:, csl], tmp)
                    else:
                        nc.vector.tensor_add(n2acc[:, csl], n2acc[:, csl],
                                             tmp)

                cmm(g_r, g_i, b1_r, b1_i, fl_t2, psp)
                _, vfull2b, _ = colsum_norms(n2acc, seq, "ch_n2",
                                             want_w=False, want_f32=False)

                # Ahat = D1^-1 B0' D0^-1 in place (so Ahat^H = A in the C
                # product below); E = B1' D2^-1 in place.
                for a in range(NAB):
                    for bx in (b0_r, b0_i):
                        nc.vector.tensor_mul(bx[:, a, :], bx[:, a, :],
                                             vfull0b)
                        nc.vector.tensor_scalar_mul(bx[:, a, :], bx[:, a, :],
                                                    w1[:, ds(a, 1)])
                    for bx in (b1_r, b1_i):
                        nc.vector.tensor_mul(bx[:, a, :], bx[:, a, :],
                                             vfull2b)

                # fence: C's flush overwrites g, still being read by T2 above
                tc.no_sync_barrier()
                # C = A @ E overwrites g (re, im)
                c_r = persist.tile([P, NAB, R], BF16, tag="g_r", name="c_r")
                c_i = persist.tile([P, NAB, R], BF16, tag="g_i", name="c_i")

                def fl_c(xb, c, ps1, ps2, ps3):
                    csl = ds(c * CW, CW)
                    nc.scalar.copy(c_r[:, xb, csl], ps1)
                    s3 = pool1.tile([P, CW], F32, tag="chc")
                    nc.scalar.copy(s3, ps3)
                    nc.vector.tensor_sub(c_i[:, xb, csl], ps2, s3)

                cmm(b0_r, b0_i, b1_r, b1_i, fl_c, psp)
                ctx2.__exit__(None, None, None)
                # fence: C.sum overwrites b0 (A), read by the C matmuls above
                tc.no_sync_barrier()
                c_s = persist.tile([P, NAB, R], BF16, tag="b0_r", name="c_s")
                for a in range(NAB):
                    nc.vector.tensor_add(c_s[:, a, :], c_r[:, a, :],
                                         c_i[:, a, :])
            return c_r, c_i, c_s

        # ---------------- pass 2: Y = x_c @ C ----------------
        def update_pass(c_r, c_i, c_s):
            with (
                tc.tile_pool(name="up_t", bufs=2) as tp_sb,
                tc.tile_pool(name="up_ps", bufs=2, space="PSUM") as tpp,
                tc.tile_pool(name="up_ops", bufs=1, space="PSUM") as opp,
                tc.tile_pool(name="up_out", bufs=2) as outp,
            ):
                for m in range(NMT):
                    qrT = tp_sb.tile([P, NAB, P], BF16, tag="qrT")
                    qiT = tp_sb.tile([P, NAB, P], BF16, tag="qiT")
                    qsT = tp_sb.tile([P, NAB, P], BF16, tag="qsT")
                    for src, dstT in ((qr_all, qrT), (qi_all, qiT)):
                        for g in range(2):
                            tp = tpp.tile([P, 4, P], BF16, tag="tp",
                                          name="tp")
                            for k in range(4):
                                nc.tensor.transpose(
                                    tp[:, k, :],
                                    src[:, m, ts(4 * g + k, P)], ident_bf)
                            nc.scalar.copy(dstT[:, ds(4 * g, 4), :], tp)
                    nc.vector.tensor_add(qsT, qrT, qiT)
                    p1 = opp.tile([P, R], F32, tag="p1", name="p1")
                    p2 = opp.tile([P, R], F32, tag="p2", name="p2")
                    p3 = opp.tile([P, R], F32, tag="p3", name="p3")
                    for n in range(NCH):
                        nsl = ds(n * CW, CW)
                        for ps, qT, cx in ((p1, qrT, c_r), (p2, qiT, c_i),
                                           (p3, qsT, c_s)):
                            for a in range(NAB):
                                nc.tensor.matmul(ps[:, nsl], qT[:, a, :],
                                                 cx[:, a, nsl],
                                                 start=(a == 0),
                                                 stop=(a == NAB - 1))
                    s2 = tp_sb.tile([P, R], F32, tag="s2")
                    nc.scalar.copy(s2, p2)
                    ot = outp.tile([P, R, 2], F32, tag="ot")
                    nc.vector.tensor_sub(ot[:, :, 0], p1, s2)
                    nc.scalar.copy(ot[:, :, 1], p3)
                    nc.vector.tensor_sub(ot[:, :, 1], ot[:, :, 1], p1)
                    nc.vector.tensor_sub(ot[:, :, 1], ot[:, :, 1], s2)
                    nc.sync.dma_start(out[ts(m, P), :, :], ot)

        rg = [list(range(N_CORES))]

        def all_reduce(dst, src):
            if single_core:
                nc.sync.dma_start(dst[:, :], src[:, :])
            else:
                nc.gpsimd.collective_compute(
                    "AllReduce", mybir.AluOpType.add, replica_groups=rg,
                    ins=[src[:, :]], outs=[dst[:, :]])

        def debug_out():
            with tc.tile_pool(name="dbg", bufs=2) as dp:
                for m in range(NMT):
                    t = dp.tile([P, R, 2], F32, tag="dbg_t")
                    nc.vector.memset(t, 0.0)
                    nc.sync.dma_start(out[ts(m, P), :, :], t)

        def _one_rep(rep):
            if rep > 0:
                # scheduler-only fence: keeps rep N+1's instructions from
                # interleaving with rep N's aliased-buffer generations (the
                # cross-rep interleave otherwise deadlocks the slot queue).
                tc.no_sync_barrier()
            if timing and rep == 0:
                with tc.tile_pool(name="seed", bufs=1) as sp:
                    ones_t = sp.tile([P, R, 2], F32, tag="seed_t")
                    nc.vector.memset(ones_t, 1.0)
                    for m in range(NMT):
                        nc.sync.dma_start(x[ts(m, P), :, :], ones_t)
                    tin = sp.tile([P, 4, 2], F32, tag="seed_x")
                    nc.sync.dma_start(tin, xin[:, :, :])
                    nc.sync.dma_start(x[0:P, 0:4, :], tin)

            gre_in = dram.tile([R, R], F32, tag=f"grei_{rep}",
                               name=f"grei_{rep}")
            t_in = dram.tile([R, R], F32, tag=f"ti_{rep}", name=f"ti_{rep}")
            gre_out = dram.tile([R, R], F32, tag=f"greo_{rep}",
                                name=f"greo_{rep}", addr_space="Shared")
            t_out = dram.tile([R, R], F32, tag=f"to_{rep}", name=f"to_{rep}",
                              addr_space="Shared")

            load_x()
            ar_gre = (lambda: all_reduce(gre_out, gre_in)) \
                if debug_stage >= 2 else (lambda: None)
            gram_pass(gre_in, t_in, ar_gre)
            if debug_stage >= 2:
                all_reduce(t_out, t_in)
            if debug_stage >= 3:
                c_parts = chain(gre_out, t_out)
            if debug_stage >= 6:
                update_pass(*c_parts)
            else:
                debug_out()

        for _rep in range(reps):
            _one_rep(_rep)

    _finalize(nc)
    return nc


def kernel(x: np.ndarray) -> np.ndarray:
    assert x.shape == (M_FULL, R, 2) and x.dtype == np.float32
    if "nc" not in _CACHE:
        _CACHE["nc"] = _build_nc()
    nc = _CACHE["nc"]
    in_maps = [{"x": np.ascontiguousarray(x[i * MS:(i + 1) * MS])}
               for i in range(N_CORES)]
    res = run_bass_kernel_spmd(nc, in_maps, core_ids=list(range(N_CORES)))
    return np.concatenate([res.results[i]["out"] for i in range(N_CORES)],
                          axis=0)
